# revision 19
# baseline (speedup 1.0000x reference)
"""CrossAttention kernel for 8 Trainium2 NeuronCores.

Sharding: batch (4) x query-row-half (2) -> 8 shards, one per core. Each core
computes the full cross-attention for its 1024 query rows of one batch:
Q/K/V projections, 8 heads of attention, and the output projection. K/V
projections are recomputed by both cores sharing a batch (20% extra flops)
in exchange for zero collectives and a pure-SPMD single NEFF.

All matmul inputs are bf16 (1 cyc/row on the PE, half the HBM traffic of
fp32; final rel-err ~5e-3 vs the 2e-2 budget). Layouts put the contraction
dim on SBUF partitions so no on-chip transposes are needed:
  KT = Wk.T @ ctxT      QT = Wq.T @ xT      V = ctxT.T @ Wv
  S_h = K_h Q_h^T       (64-row PE tiles T0/T8, head pairs packed)
  P = exp(S * scale)    (Scalar engine; no max-subtraction, logits ~N(0,1))
  O^T_h | den_h = [V_h | ones].T @ P   (denominator rides in the M dim)
  Y = (O^T/den).T @ Wo + bo

The Scalar engine's exp is the co-bottleneck (~71us vs ~90us of PE matmul),
so the schedule interleaves one PV/projection filler block after every
score group, keeping the PE busy exactly while exp catches up. Softmax
normalization runs directly off PSUM with a cross-quadrant DVE reciprocal
(the denominator comes out replicated on the other 64 partitions).
PSUM: score/proj groups 3x(2 banks) + 2 PV accumulators = 8 banks.
"""

import numpy as np

HEADS = 8
DIM_HEAD = 64
SCALE = DIM_HEAD ** -0.5
B, NQ, DQ = 4, 2048, 512
NK, DC = 1024, 768
INNER = HEADS * DIM_HEAD  # 512
NQH = NQ // 2             # query rows per core
N_CORES = 8
P = 128

_PROG_CACHE = {}


def _build_program():
    import concourse.bacc as bacc
    import concourse.tile as tile
    from concourse import mybir
    from concourse.bass import ts, ds

    f32 = mybir.dt.float32
    bf16 = mybir.dt.bfloat16
    Exp = mybir.ActivationFunctionType.Exp

    nc = bacc.Bacc(
        "TRN2",
        target_bir_lowering=False,
        debug=False,
        num_devices=N_CORES,
    )

    xT_d = nc.dram_tensor("xT", [DQ, NQH], bf16, kind="ExternalInput")
    ctxT_d = nc.dram_tensor("ctxT", [DC, NK], bf16, kind="ExternalInput")
    Wq_d = nc.dram_tensor("Wq", [DQ, INNER], bf16, kind="ExternalInput")
    Wk_d = nc.dram_tensor("Wk", [DC, INNER], bf16, kind="ExternalInput")
    Wv_d = nc.dram_tensor("Wv", [DC, INNER], bf16, kind="ExternalInput")
    Wo_d = nc.dram_tensor("Wo", [INNER, DQ], bf16, kind="ExternalInput")
    bo_d = nc.dram_tensor("bo", [DQ], f32, kind="ExternalInput")
    ones_d = nc.dram_tensor("ones", [4, 128], bf16, kind="ExternalInput")
    Y_d = nc.dram_tensor("Y", [NQH, DQ], f32, kind="ExternalOutput")

    KQ = DQ // P      # 4  k-tiles for x-side contraction
    KC = DC // P      # 6  k-tiles for context-side contraction
    KI = INNER // P   # 4  k-tiles for inner-dim contraction (= head pairs)
    NKT = NK // P     # 8  key row tiles
    NCH = NQH // 512  # 2  nq chunks of 512

    with tile.TileContext(nc) as tc:
        with (
            tc.tile_pool(name="consts", bufs=1) as consts,
            tc.tile_pool(name="st", bufs=3, space="PSUM") as stp,
            tc.tile_pool(name="po", bufs=2, space="PSUM") as pop,
            tc.tile_pool(name="ep", bufs=16) as ep,
            tc.tile_pool(name="rec", bufs=2) as recp,
            tc.tile_pool(name="yp", bufs=2) as yp,
            tc.tile_pool(name="dram", bufs=2, space="DRAM") as dramp,
        ):
            # ---- staged inputs: fine-grained DMAs issued in exact
            # consumption order, round-robined over four issuing engines so
            # arrival order tracks issue order (transfers are HBM-limited) ----
            Wk_sb = consts.tile([P, KC, INNER], bf16, tag="wk")
            ctx_sb = consts.tile([P, KC, NK], bf16, tag="ctx")
            Wq_sb = consts.tile([P, KQ, INNER], bf16, tag="wq")
            xT_sb = consts.tile([P, KQ, NQH], bf16, tag="x")
            Wv_sb = consts.tile([P, KC, INNER], bf16, tag="wv")
            V_sb = consts.tile([P, NKT, HEADS * P], bf16, tag="v")
            Wo_sb = consts.tile([P, KI, DQ], bf16, tag="wo")
            bo_sb = consts.tile([P, DQ], f32, tag="bo")

            ctx_src = ctxT_d.ap().rearrange("(ko p) n -> p ko n", p=P)
            xT_src = xT_d.ap().rearrange("(ko p) n -> p ko n", p=P)
            Wk_src = Wk_d.ap().rearrange("(ko p) i -> p ko i", p=P)
            Wq_src = Wq_d.ap().rearrange("(ko p) i -> p ko i", p=P)
            ones_src = ones_d.ap().unsqueeze(0).to_broadcast((P, 4, 128))

            engs = [nc.gpsimd, nc.sync, nc.scalar]
            qi = [0]

            def ld(out_ap, in_ap, nengs=3):
                engs[qi[0] % nengs].dma_start(out=out_ap, in_=in_ap)
                qi[0] += 1

            # Kproj(0,c0) path first: Wk j0 column slice + ctx c0 k-slices
            ld(Wk_sb[:, :, 0:P], Wk_src[:, :, 0:P])
            for k in range(KC):
                ld(ctx_sb[:, k:k + 1, 0:512], ctx_src[:, k:k + 1, 0:512])
            # then Kproj(0,c1) and Qproj(0,*) deps
            for k in range(KC):
                ld(ctx_sb[:, k:k + 1, 512:1024], ctx_src[:, k:k + 1, 512:1024])
            ld(Wq_sb[:, :, 0:P], Wq_src[:, :, 0:P])
            for k in range(KQ):
                ld(xT_sb[:, k:k + 1, 0:512], xT_src[:, k:k + 1, 0:512])
            for k in range(KQ):
                ld(xT_sb[:, k:k + 1, 512:1024], xT_src[:, k:k + 1, 512:1024])
            # V projection + later head pairs; scalar drops out (exp starts)
            ld(Wv_sb, Wv_d.ap().rearrange("(ko p) i -> p ko i", p=P), nengs=2)
            ld(Wk_sb[:, :, P:INNER], Wk_src[:, :, P:INNER], nengs=2)
            ld(Wq_sb[:, :, P:INNER], Wq_src[:, :, P:INNER], nengs=2)
            # V padding: even head h has ones in cols [64:128] of its block,
            # odd head in [0:64]; merged regions = cols 64:192 per pair block.
            for t in range(NKT):
                dv4 = V_sb[:, t, :].rearrange("p (j y) -> p j y", j=4)
                ld(dv4[:, :, 64:192], ones_src, nengs=2)
            ld(Wo_sb, Wo_d.ap().rearrange("(ko p) i -> p ko i", p=P), nengs=2)
            ld(bo_sb, bo_d.ap().unsqueeze(0).to_broadcast((P, DQ)), nengs=2)

            KT_sb = consts.tile([P, KI, NK], bf16, tag="kt")   # [i, nk]
            QT_sb = consts.tile([P, KI, NQH], bf16, tag="qt")  # [i, nq]
            OT_sb = consts.tile([P, KI, NQH], bf16, tag="ot")  # [i, nq] norm'd

            # ---- emission helpers ----
            def emit_kproj(j, c):
                psk = stp.tile([P, 2, 512], f32, tag="st", name=f"psk{j}{c}")
                for k in range(KC):
                    nc.tensor.matmul(
                        psk[:, 0, :], lhsT=Wk_sb[:, k, ts(j, P)],
                        rhs=ctx_sb[:, k, ds(c * 512, 512)],
                        start=(k == 0), stop=(k == KC - 1),
                    )
                nc.vector.tensor_copy(
                    KT_sb[:, j, ds(c * 512, 512)], psk[:, 0, :])

            def emit_qproj(j, c):
                psq = stp.tile([P, 2, 512], f32, tag="st", name=f"psq{j}{c}")
                for k in range(KQ):
                    nc.tensor.matmul(
                        psq[:, 0, :], lhsT=Wq_sb[:, k, ts(j, P)],
                        rhs=xT_sb[:, k, ds(c * 512, 512)],
                        start=(k == 0), stop=(k == KQ - 1),
                    )
                nc.vector.tensor_copy(
                    QT_sb[:, j, ds(c * 512, 512)], psq[:, 0, :])

            def emit_vproj(t):
                psv = stp.tile([P, 2, 512], f32, tag="st", name=f"psv{t}")
                for k in range(KC):
                    nc.tensor.matmul(
                        psv[:, 0, :], lhsT=ctx_sb[:, k, ts(t, P)],
                        rhs=Wv_sb[:, k, :],
                        start=(k == 0), stop=(k == KC - 1),
                    )
                pv4 = psv[:, 0, :].rearrange("p (j x) -> p j x", j=4)
                dv4 = V_sb[:, t, :].rearrange("p (j y) -> p j y", j=4)
                nc.vector.tensor_copy(dv4[:, :, 0:64], pv4[:, :, 0:64])
                nc.vector.tensor_copy(dv4[:, :, 192:256], pv4[:, :, 64:128])

            # score group g of (j, c): t-tiles {2g, 2g+1} for both heads of
            # pair j. A = head 2j (rows 0:64, PE tile T0), B = head 2j+1
            # (rows 64:128, T8); adjacent T0/T8 matmuls can overlap on
            # disjoint PE row-halves.
            e_tiles = {}

            def emit_st_group(j, c, g):
                psA = stp.tile([P, 2, 512], f32, tag="st", name=f"psA{j}{c}{g}")
                psB = stp.tile([P, 2, 512], f32, tag="st", name=f"psB{j}{c}{g}")
                for i, t in enumerate((2 * g, 2 * g + 1)):
                    nc.tensor.matmul(
                        psA[:, i, :], lhsT=KT_sb[0:64, j, ts(t, P)],
                        rhs=QT_sb[0:64, j, ds(c * 512, 512)],
                        start=True, stop=True,
                    )
                    nc.tensor.matmul(
                        psB[:, i, :], lhsT=KT_sb[64:128, j, ts(t, P)],
                        rhs=QT_sb[64:128, j, ds(c * 512, 512)],
                        start=True, stop=True,
                    )
                eA = ep.tile([P, 2, 512], bf16, tag="e", name=f"eA{j}{c}{g}")
                eB = ep.tile([P, 2, 512], bf16, tag="e", name=f"eB{j}{c}{g}")
                nc.scalar.activation(out=eA, in_=psA, func=Exp, scale=SCALE)
                nc.scalar.activation(out=eB, in_=psB, func=Exp, scale=SCALE)
                e_tiles[(j, c, g, 0)] = eA
                e_tiles[(j, c, g, 1)] = eB

            po_tiles = {}

            def emit_pv_group(j, c, g):
                if g == 0:
                    po_tiles[0] = pop.tile([P, 512], f32, tag="po",
                                           name=f"po{j}{c}a")
                    po_tiles[1] = pop.tile([P, 512], f32, tag="po",
                                           name=f"po{j}{c}b")
                for ab in range(2):
                    h = 2 * j + ab
                    po = po_tiles[ab]
                    e = e_tiles.pop((j, c, g, ab))
                    for i, t in enumerate((2 * g, 2 * g + 1)):
                        nc.tensor.matmul(
                            po, lhsT=V_sb[:, t, ds(h * P, P)], rhs=e[:, i, :],
                            start=(t == 0), stop=(t == NKT - 1),
                        )
                if g == 3:
                    for ab in range(2):
                        _norm_head(j, c, 2 * j + ab, po_tiles[ab])

            def _norm_head(j, c, h, po):
                # evict PSUM fast; normalize off SBUF. The denominator comes
                # out replicated on the ones-partitions; chop one row to
                # [64, 8] for a cheap DVE reciprocal (DVE recip is ~6.5
                # ns/free-elem, so never run it 512 wide), then broadcast
                # 1/den back across partitions via a DRAM bounce. Head A's
                # chain runs on gpsimd, head B's on sync, so back-to-back
                # norms don't serialize on one DMA queue.
                dmae = nc.gpsimd if h % 2 == 0 else nc.sync
                o_raw = recp.tile([P, 512], f32, tag="oraw",
                                  name=f"oraw{j}{c}{h}")
                nc.vector.tensor_copy(o_raw, po)
                olo, ohi = (0, 64) if h % 2 == 0 else (64, 128)
                dlo = 64 if h % 2 == 0 else 0
                dg = recp.tile([64, 8], f32, tag="dg", name=f"dg{j}{c}{h}")
                dmae.dma_start(out=dg, in_=o_raw[dlo:dlo + 1, :])
                rg = recp.tile([64, 8], f32, tag="rg", name=f"rg{j}{c}{h}")
                nc.vector.reciprocal(rg, dg)
                dsc = dramp.tile([512], f32, tag="ds", name=f"ds{j}{c}{h}")
                dmae.dma_start(out=dsc, in_=rg)
                rb = recp.tile([P, 512], f32, tag="rb", name=f"rb{j}{c}{h}")
                dmae.dma_start(
                    out=rb[olo:ohi, :],
                    in_=dsc.unsqueeze(0).to_broadcast((64, 512)),
                )
                nc.vector.tensor_tensor(
                    OT_sb[olo:ohi, j, ds(c * 512, 512)],
                    o_raw[olo:ohi, :], rb[olo:ohi, :],
                    op=mybir.AluOpType.mult,
                )

            def emit_yproj(m):
                psy = stp.tile([P, 2, 512], f32, tag="st", name=f"psy{m}")
                for k in range(KI):
                    nc.tensor.matmul(
                        psy[:, 0, :], lhsT=OT_sb[:, k, ts(m, P)],
                        rhs=Wo_sb[:, k, :],
                        start=(k == 0), stop=(k == KI - 1),
                    )
                y_t = yp.tile([P, DQ], f32, tag="y", name=f"y{m}")
                nc.vector.tensor_tensor(y_t, psy[:, 0, :], bo_sb,
                                        op=mybir.AluOpType.add)
                nc.sync.dma_start(out=Y_d.ap()[ts(m, P), :], in_=y_t)

            # ---- schedule ----
            # Iteration order (c0: j=0..3), (c1: j=0..3); each iteration's
            # score groups interleave with the PREVIOUS iteration's PV blocks
            # plus projection dribbles, so the PE always has ~2x the Scalar
            # engine's exp time in queued work and pair 3's first-half PV
            # lands mid-kernel instead of at the tail.
            emit_kproj(0, 0)
            emit_kproj(0, 1)
            emit_qproj(0, 0)

            seq = [(j, 0) for j in range(KI)] + [(j, 1) for j in range(KI)]
            for i, (j, c) in enumerate(seq):
                prev = seq[i - 1] if i > 0 else None
                for g in range(4):
                    emit_st_group(j, c, g)
                    if prev is not None:
                        emit_pv_group(prev[0], prev[1], g)
                    if i == 0:
                        emit_vproj(2 * g)
                        emit_vproj(2 * g + 1)
                    if c == 0:
                        if g == 0:
                            emit_qproj(j, 1)
                        elif g == 1 and j + 1 < KI:
                            emit_kproj(j + 1, 0)
                        elif g == 2 and j + 1 < KI:
                            emit_qproj(j + 1, 0)
                        elif g == 3 and j + 1 < KI:
                            emit_kproj(j + 1, 1)
                    else:
                        # late c1 iterations: dribble one Y(c0) m-tile;
                        # the rest stay at the tail to cover the final
                        # normalization chain (pair 3 chunk c0 was
                        # normalized back at iteration 4)
                        if j == 2 and g == 1:
                            emit_yproj(0)

            # tail: PV of (3, c1); Y(c0) m1..m3 (~2.8us of ready matmuls)
            # cover the final norm chain before Y(c1)'s k=3 needs it.
            for g in range(4):
                emit_pv_group(KI - 1, 1, g)
            for m in (1, 2, 3):
                emit_yproj(m)
            for m in range(4):
                emit_yproj(4 + m)

    nc.finalize()
    return nc


def _get_program():
    if "nc" not in _PROG_CACHE:
        _PROG_CACHE["nc"] = _build_program()
    return _PROG_CACHE["nc"]


def _bf16(a):
    import ml_dtypes
    return np.ascontiguousarray(a).astype(ml_dtypes.bfloat16)


def _ones_bf16():
    import ml_dtypes
    return np.ones((4, 128), dtype=ml_dtypes.bfloat16)


def _build_in_maps(x, context, Wq, Wk, Wv, Wo, bo):
    x = np.asarray(x, dtype=np.float32)
    context = np.asarray(context, dtype=np.float32)
    Wq_b = _bf16(np.asarray(Wq, dtype=np.float32))
    Wk_b = _bf16(np.asarray(Wk, dtype=np.float32))
    Wv_b = _bf16(np.asarray(Wv, dtype=np.float32))
    Wo_b = _bf16(np.asarray(Wo, dtype=np.float32))
    bo_f = np.ascontiguousarray(np.asarray(bo, dtype=np.float32))
    ones = _ones_bf16()
    in_maps = []
    for core in range(N_CORES):
        b, half = divmod(core, 2)
        xs = _bf16(x[b, half * NQH:(half + 1) * NQH, :].T)
        cs = _bf16(context[b].T)
        in_maps.append(
            {"xT": xs, "ctxT": cs, "Wq": Wq_b, "Wk": Wk_b, "Wv": Wv_b,
             "Wo": Wo_b, "bo": bo_f, "ones": ones}
        )
    return in_maps


def kernel(x, context, Wq, Wk, Wv, Wo, bo, **_unused):
    from concourse.bass_utils import run_bass_kernel_spmd

    nc = _get_program()
    in_maps = _build_in_maps(x, context, Wq, Wk, Wv, Wo, bo)
    res = run_bass_kernel_spmd(nc, in_maps, core_ids=list(range(N_CORES)))

    out = np.empty((B, NQ, DQ), np.float32)
    for core in range(N_CORES):
        b, half = divmod(core, 2)
        out[b, half * NQH:(half + 1) * NQH, :] = res.results[core]["Y"]
    return out
